# revision 6
# baseline (speedup 1.0000x reference)
"""Trainium2 Bass kernel for nn_Encoder_52312701666158 (dense-GCN encoder).

Math (per graph):
    x   = concat(type_emb[types], label_emb[labels])          [N, 64]
    deg = clip(adj.sum(-1), 1, inf); dis = deg**-0.5
    H1  = relu(dis*(adj @ (dis*(x@W1))) + b1)
    H2  = relu(dis*(adj @ (dis*(H1@W2))) + b2)
    out = concat(H2.mean(0), H2.max(0)) @ Wr.T + br           [64]

Sharding: data-parallel over the batch dim, 2 graphs per NeuronCore x 8 cores.

Per-core device strategy (DMA-roofline oriented; adj is ~67MB/graph and must
be read 3x: deg pass, layer-1, layer-2):
  Pass 1   read adj naturally (16KB rows, full DMA bandwidth), reduce rows
           on DVE for deg, and write adj back to a DRAM scratch as fp16
           (cast done inside the SWDGE DMA, zero engine cost).
  Layers   the tensor engine contracts along the partition dim, so adj must
           be fed TRANSPOSED; hardware xbar DMA-transpose (fp16-only) loads
           A.T tiles straight from the fp16 scratch.  lhsT = Z[j-tile]
           (stationary), rhs = A.T[j, i-chunk], accumulating Y.T = (A@Z).T
           in PSUM f32 over all j.
  Embeddings are folded into gather tables (U_t = type_emb@W1[:32],
           U_l = label_emb@W1[32:]) built on-device once, then row-gathered
           per node via indirect DMA; x@W1 = U_t[types] + U_l[labels].
  fp16 is used only for adj and Z in the big matmuls (PSUM accumulates f32);
  everything else stays f32.
"""

import numpy as np

import concourse.bass as bass
import concourse.bacc as bacc
import concourse.mybir as mybir
import concourse.tile as tile
from concourse import bass_utils
from concourse.masks import make_identity

B, N, D = 16, 4096, 64
NCORES = 8
BPC = B // NCORES          # graphs per core
NT = N // 128              # node tiles per graph
HALF = 2048                # i-chunk span per PSUM accumulator (4 banks)
NH = N // HALF
VOCAB, NTYPES, EMB = 1000, 16, 32

F32 = mybir.dt.float32
FP16 = mybir.dt.float16
I32 = mybir.dt.int32

_CACHE = {}


def _build(BPC=BPC, N=N, HALF=HALF, NCORES=NCORES, STAGE=99):
    NT = N // 128
    NH = N // HALF
    nc = bacc.Bacc("TRN2", target_bir_lowering=False, debug=False, num_devices=NCORES)

    nt_in = nc.dram_tensor("node_types", [BPC, N], I32, kind="ExternalInput").ap()
    lb_in = nc.dram_tensor("node_labels", [BPC, N], I32, kind="ExternalInput").ap()
    adj = nc.dram_tensor("adj", [BPC, N, N], F32, kind="ExternalInput").ap()
    temb = nc.dram_tensor("type_emb", [NTYPES, EMB], F32, kind="ExternalInput").ap()
    lemb = nc.dram_tensor("label_emb", [VOCAB, EMB], F32, kind="ExternalInput").ap()
    w1 = nc.dram_tensor("W1", [D, D], F32, kind="ExternalInput").ap()
    b1 = nc.dram_tensor("b1", [D], F32, kind="ExternalInput").ap()
    w2 = nc.dram_tensor("W2", [D, D], F32, kind="ExternalInput").ap()
    b2 = nc.dram_tensor("b2", [D], F32, kind="ExternalInput").ap()
    wr = nc.dram_tensor("Wr", [D, 2 * D], F32, kind="ExternalInput").ap()
    br = nc.dram_tensor("br", [D], F32, kind="ExternalInput").ap()
    out = nc.dram_tensor("out", [BPC, D], F32, kind="ExternalOutput").ap()

    with tile.TileContext(nc) as tc:
        with (
            tc.tile_pool(name="consts", bufs=1) as consts,
            tc.tile_pool(name="dram", bufs=1, space="DRAM") as dpool,
            tc.tile_pool(name="strips", bufs=3) as strips,
            tc.tile_pool(name="gstate", bufs=2) as gstate,
            tc.tile_pool(name="zpool", bufs=2) as zpool,
            tc.tile_pool(name="work", bufs=3) as work,
            tc.tile_pool(name="rpool", bufs=3) as rpool,
            tc.tile_pool(name="ytp", bufs=2) as ytp,
            tc.tile_pool(name="accp", bufs=1, space="PSUM") as accp,
            tc.tile_pool(name="pssp", bufs=3, space="PSUM") as pssp,
        ):
            def ps_small(shape, name):
                return pssp.tile(shape, F32, tag="pss", name=name)

            # ---------------- Phase 0: constants + gather tables ----------------
            ident = consts.tile([128, 128], F32)
            make_identity(nc, ident[:])

            w1a = consts.tile([EMB, D], F32)
            nc.sync.dma_start(out=w1a[:], in_=w1[0:EMB, :])
            w1b = consts.tile([EMB, D], F32)
            nc.sync.dma_start(out=w1b[:], in_=w1[EMB:D, :])
            w2s = consts.tile([D, D], F32)
            nc.sync.dma_start(out=w2s[:], in_=w2[:, :])
            wrs = consts.tile([D, 2 * D], F32)
            nc.sync.dma_start(out=wrs[:], in_=wr[:, :])
            brs = consts.tile([1, D], F32)
            nc.sync.dma_start(out=brs[:], in_=br[None, :])

            def bcast128(src_ap, name):
                t = consts.tile([128, D], F32, name=name)
                bc = bass.AP(tensor=src_ap.tensor, offset=src_ap.offset,
                             ap=[[0, 128]] + list(src_ap.ap))
                nc.gpsimd.dma_start(out=t[:], in_=bc)
                return t

            b1r = bcast128(b1, "b1r")
            b2r = bcast128(b2, "b2r")

            # Wr.T halves for the readout matmul
            wrmT = consts.tile([D, D], F32)
            wrxT = consts.tile([D, D], F32)
            for half, dst in ((0, wrmT), (1, wrxT)):
                tps = ps_small([D, D], f"wrt_ps{half}")
                nc.tensor.transpose(out=tps[:], in_=wrs[:, half * D:(half + 1) * D],
                                    identity=ident[:D, :D])
                nc.scalar.copy(out=dst[:], in_=tps[:])

            # U_t = type_emb @ W1[:32]  -> [16, 64] DRAM table
            ut_tab = dpool.tile([NTYPES, D], F32)
            tes = consts.tile([NTYPES, EMB], F32)
            nc.sync.dma_start(out=tes[:], in_=temb[:, :])
            teT_ps = ps_small([EMB, NTYPES], "teT_ps")
            nc.tensor.transpose(out=teT_ps[:], in_=tes[:], identity=ident[:NTYPES, :NTYPES])
            teT = consts.tile([EMB, NTYPES], F32)
            nc.scalar.copy(out=teT[:], in_=teT_ps[:])
            ut_ps = ps_small([NTYPES, D], "ut_ps")
            nc.tensor.matmul(out=ut_ps[:], lhsT=teT[:], rhs=w1a[:], start=True, stop=True)
            ut_sb = consts.tile([NTYPES, D], F32)
            nc.scalar.copy(out=ut_sb[:], in_=ut_ps[:])
            nc.sync.dma_start(out=ut_tab[:, :], in_=ut_sb[:])

            # U_l = label_emb @ W1[32:]  -> [1000, 64] DRAM table
            ul_tab = dpool.tile([VOCAB, D], F32)
            for k in range((VOCAB + 127) // 128):
                r0 = k * 128
                nr = min(128, VOCAB - r0)
                les = work.tile([128, EMB], F32, tag="les", name=f"les{k}")
                nc.sync.dma_start(out=les[:nr, :], in_=lemb[r0:r0 + nr, :])
                leT_ps = ps_small([EMB, 128], f"leT_ps{k}")
                nc.tensor.transpose(out=leT_ps[:, :nr], in_=les[:nr, :],
                                    identity=ident[:nr, :nr])
                leT = work.tile([EMB, 128], F32, tag="leT", name=f"leT{k}")
                nc.scalar.copy(out=leT[:, :nr], in_=leT_ps[:, :nr])
                ul_ps = ps_small([128, D], f"ul_ps{k}")
                nc.tensor.matmul(out=ul_ps[:nr, :], lhsT=leT[:, :nr], rhs=w1b[:],
                                 start=True, stop=True)
                ul_sb = work.tile([128, D], F32, tag="ul_sb", name=f"ul_sb{k}")
                nc.scalar.copy(out=ul_sb[:nr, :], in_=ul_ps[:nr, :])
                nc.sync.dma_start(out=ul_tab[r0:r0 + nr, :], in_=ul_sb[:nr, :])

            # ---------------- Per-graph pipeline ----------------
            for g in range(BPC):
                a16 = dpool.tile([N, N], FP16, tag=f"a16_{g}", name=f"a16_{g}")

                # ---- Pass 1: deg (row sums) + fp16 cast-store of adj
                deg = gstate.tile([128, NT], F32, tag="deg", name=f"deg{g}")
                for s in range(NT):
                    strip = strips.tile([128, N], F32, tag="strip", name=f"st{g}_{s}")
                    nc.sync.dma_start(out=strip[:], in_=adj[g, s * 128:(s + 1) * 128, :])
                    nc.vector.reduce_sum(out=deg[:, s:s + 1], in_=strip[:],
                                         axis=mybir.AxisListType.X)
                    nc.gpsimd.dma_start(out=a16[s * 128:(s + 1) * 128, :], in_=strip[:])

                dis = gstate.tile([128, NT], F32, tag="dis", name=f"dis{g}")
                nc.vector.tensor_scalar_max(deg[:], deg[:], 1.0)
                nc.scalar.activation(out=deg[:], in_=deg[:],
                                     func=mybir.ActivationFunctionType.Sqrt)
                nc.vector.reciprocal(out=dis[:], in_=deg[:])
                if STAGE <= 1:
                    nc.sync.dma_start(out=out[g:g + 1, 0:NT], in_=dis[0:1, :])
                    continue

                # ---- Z1 = dis * (U_t[types] + U_l[labels]), node-major fp16 tiles
                z1 = []
                for t in range(NT):
                    tyi = work.tile([128, 1], I32, tag="tyi", name=f"tyi{g}_{t}")
                    nc.sync.dma_start(out=tyi[:], in_=nt_in[g, t * 128:(t + 1) * 128, None])
                    lbi = work.tile([128, 1], I32, tag="lbi", name=f"lbi{g}_{t}")
                    nc.sync.dma_start(out=lbi[:], in_=lb_in[g, t * 128:(t + 1) * 128, None])
                    gt = work.tile([128, D], F32, tag="gt", name=f"gt{g}_{t}")
                    nc.gpsimd.indirect_dma_start(
                        out=gt[:], out_offset=None, in_=ut_tab[:, :],
                        in_offset=bass.IndirectOffsetOnAxis(ap=tyi[:, :1], axis=0))
                    gl = work.tile([128, D], F32, tag="gl", name=f"gl{g}_{t}")
                    nc.gpsimd.indirect_dma_start(
                        out=gl[:], out_offset=None, in_=ul_tab[:, :],
                        in_offset=bass.IndirectOffsetOnAxis(ap=lbi[:, :1], axis=0))
                    zt = zpool.tile([128, D], FP16, tag=f"z1_{t}", name=f"z1_{g}_{t}")
                    nc.vector.tensor_add(out=gt[:], in0=gt[:], in1=gl[:])
                    nc.vector.tensor_scalar_mul(gt[:], gt[:], dis[:, t:t + 1])
                    nc.vector.tensor_copy(out=zt[:], in_=gt[:])
                    z1.append(zt)
                if STAGE <= 2:
                    nc.gpsimd.dma_start(out=out[g:g + 1, :], in_=z1[0][0:1, :])
                    continue

                # ---- Two GCN layers
                zs = z1
                h2T = gstate.tile([D, N], F32, tag="h2T", name=f"h2T{g}")
                for ell in range(min(2, STAGE - 2)):
                    brep = b1r if ell == 0 else b2r
                    yt = ytp.tile([D, N], F32, tag="yt", name=f"yt{g}_{ell}")
                    for h in range(NH):
                        acc = accp.tile([D, HALF], F32, tag="acc", name=f"acc{g}_{ell}_{h}")
                        for jt in range(NT):
                            rhs = rpool.tile([128, HALF], FP16, tag="rhs",
                                             name=f"rhs{g}_{ell}_{h}_{jt}")
                            nc.scalar.dma_start(
                                out=rhs[:],
                                in_=a16[h * HALF:(h + 1) * HALF, jt * 128:(jt + 1) * 128],
                                transpose=True)
                            for c in range(HALF // 512):
                                nc.tensor.matmul(
                                    out=acc[:, c * 512:(c + 1) * 512],
                                    lhsT=zs[jt][:],
                                    rhs=rhs[:, c * 512:(c + 1) * 512],
                                    start=(jt == 0), stop=(jt == NT - 1))
                        # drain PSUM -> SBUF (split across DVE and ACT)
                        nc.vector.tensor_copy(out=yt[:, h * HALF:h * HALF + HALF // 2],
                                              in_=acc[:, :HALF // 2])
                        nc.scalar.copy(out=yt[:, h * HALF + HALF // 2:(h + 1) * HALF],
                                       in_=acc[:, HALF // 2:])

                    # ---- post-process: V -> node-major, H = relu(dis*V + b)
                    znext = []
                    for t in range(NT):
                        vps = ps_small([128, D], f"vps{g}_{ell}_{t}")
                        nc.tensor.transpose(out=vps[:], in_=yt[:, t * 128:(t + 1) * 128],
                                            identity=ident[:D, :D])
                        hb = work.tile([128, D], F32, tag="hb", name=f"hb{g}_{ell}_{t}")
                        nc.vector.tensor_scalar_mul(hb[:], vps[:], dis[:, t:t + 1])
                        nc.vector.tensor_add(out=hb[:], in0=hb[:], in1=brep[:])
                        nc.vector.tensor_scalar_max(hb[:], hb[:], 0.0)
                        if ell == 0:
                            # Z2 tile = (dis*H1) @ W2, cast fp16
                            nc.vector.tensor_scalar_mul(hb[:], hb[:], dis[:, t:t + 1])
                            qtps = ps_small([D, 128], f"qtps{g}_{t}")
                            nc.tensor.transpose(out=qtps[:], in_=hb[:], identity=ident[:])
                            qT = work.tile([D, 128], F32, tag="qT", name=f"qT{g}_{t}")
                            nc.scalar.copy(out=qT[:], in_=qtps[:])
                            z2ps = ps_small([128, D], f"z2ps{g}_{t}")
                            nc.tensor.matmul(out=z2ps[:], lhsT=qT[:], rhs=w2s[:],
                                             start=True, stop=True)
                            zt2 = zpool.tile([128, D], FP16, tag=f"z2_{t}",
                                             name=f"z2_{g}_{t}")
                            nc.scalar.copy(out=zt2[:], in_=z2ps[:])
                            znext.append(zt2)
                        else:
                            h2tps = ps_small([D, 128], f"h2tps{g}_{t}")
                            nc.tensor.transpose(out=h2tps[:], in_=hb[:], identity=ident[:])
                            nc.scalar.copy(out=h2T[:, t * 128:(t + 1) * 128], in_=h2tps[:])
                    zs = znext
                if STAGE <= 3:
                    nc.gpsimd.dma_start(out=out[g:g + 1, :], in_=zs[0][0:1, :])
                    continue
                if STAGE <= 4:
                    nc.sync.dma_start(out=out[g:g + 1, :], in_=h2T[0:1, 0:D])
                    continue

                # ---- pooling + readout
                sums = work.tile([D, 1], F32, tag="sums", name=f"sums{g}")
                nc.vector.reduce_sum(out=sums[:], in_=h2T[:], axis=mybir.AxisListType.X)
                nc.vector.tensor_scalar_mul(sums[:], sums[:], 1.0 / N)
                mx = work.tile([D, 1], F32, tag="mx", name=f"mx{g}")
                nc.vector.reduce_max(out=mx[:], in_=h2T[:], axis=mybir.AxisListType.X)
                ops = ps_small([1, D], f"ops{g}")
                nc.tensor.matmul(out=ops[:], lhsT=sums[:], rhs=wrmT[:], start=True, stop=False)
                nc.tensor.matmul(out=ops[:], lhsT=mx[:], rhs=wrxT[:], start=False, stop=True)
                ob = work.tile([1, D], F32, tag="ob", name=f"ob{g}")
                nc.vector.tensor_add(out=ob[:], in0=ops[:], in1=brs[:])
                nc.sync.dma_start(out=out[g:g + 1, :], in_=ob[:])

    nc.compile()
    return nc


def _get_program():
    if "nc" not in _CACHE:
        _CACHE["nc"] = _build()
    return _CACHE["nc"]


def _shard_inputs(inputs):
    f32 = np.float32
    i32 = np.int32
    nt = np.ascontiguousarray(np.asarray(inputs["node_types"], dtype=i32))
    lb = np.ascontiguousarray(np.asarray(inputs["node_labels"], dtype=i32))
    adj = np.asarray(inputs["adj"], dtype=f32)
    rep = {
        "type_emb": np.ascontiguousarray(np.asarray(inputs["type_emb"], dtype=f32)),
        "label_emb": np.ascontiguousarray(np.asarray(inputs["label_emb"], dtype=f32)),
        "W1": np.ascontiguousarray(np.asarray(inputs["W1"], dtype=f32)),
        "b1": np.ascontiguousarray(np.asarray(inputs["b1"], dtype=f32)),
        "W2": np.ascontiguousarray(np.asarray(inputs["W2"], dtype=f32)),
        "b2": np.ascontiguousarray(np.asarray(inputs["b2"], dtype=f32)),
        "Wr": np.ascontiguousarray(np.asarray(inputs["Wr"], dtype=f32)),
        "br": np.ascontiguousarray(np.asarray(inputs["br"], dtype=f32)),
    }
    in_maps = []
    for c in range(NCORES):
        s = slice(c * BPC, (c + 1) * BPC)
        in_maps.append({
            "node_types": nt[s],
            "node_labels": lb[s],
            "adj": np.ascontiguousarray(adj[s]),
            **rep,
        })
    return in_maps


def run_sharded(inputs, trace=False, **kw):
    """Returns (output [B, D] f32, BassKernelResults)."""
    nc = _get_program()
    in_maps = _shard_inputs(inputs)
    res = bass_utils.run_bass_kernel_spmd(nc, in_maps, core_ids=list(range(NCORES)),
                                          trace=trace, **kw)
    outp = np.concatenate([res.results[c]["out"] for c in range(NCORES)], axis=0)
    return outp.astype(np.float32), res


def kernel(**inputs) -> np.ndarray:
    outp, _ = run_sharded(inputs, trace=False)
    return outp
